# revision 1
# baseline (speedup 1.0000x reference)
"""Masked dot-product attention (B=2,H=16,L=2048,D=128) on 8 trn2 NeuronCores.

Strategy:
  - Shard batch*heads: core c handles (b=0,h=2c),(0,2c+1),(1,2c),(1,2c+1) -> 4 slots.
  - Per (b,h): compute S^T[k,q] = K Q^T directly on the PE (lhsT = k-tile
    transposed to [D,k], rhs = q transposed to [D,q]) so softmax masking is a
    per-partition bias on the exp eviction, and no P-transposes are needed.
  - Only ceil(valid_len/128) key tiles are computed (the rest contribute
    exactly 0 after exp of -1e9, matching the reference's mask fill).
  - exp is fused into the PSUM->SBUF eviction on the scalar engine with
    scale = 1/sqrt(D); j pairs share one exp instruction. The last (partial)
    key tile gets a per-partition -1e9 bias.
  - O^T[d,q] += V_j^T P^T_j accumulates in PSUM (fp32); the softmax
    denominator l accumulates via an all-ones [128,1] lhsT matmul into a
    [1,512] PSUM row per q block.
  - l is transposed into per-partition layout via a DRAM bounce (or tiny K=1
    matmuls for the tail slot, avoiding the DMA latency), reciprocal on the
    DVE, then O^T transposes back to [q,d] on the PE with the final eviction
    scaled by 1/l per partition. Per-slot finish phases are deferred by one
    slot / one q block so their latency hides under later compute. Hot
    matmuls use float32r (~12-bit mantissa, 4x fp32 PE throughput);
    accumulation stays fp32 in PSUM. q rows are processed in a
    (p t)-permuted order so q loads and output stores use contiguous 2KB
    DMA descriptors; the permutation is applied consistently to l and O.
"""

import math

import numpy as np

try:
    import concourse.bass as bass
except ImportError:  # pragma: no cover
    import sys

    sys.path.append("/opt/trn_rl_repo")
    import concourse.bass as bass

import concourse.mybir as mybir
import concourse.tile as tile
from concourse import bacc
from concourse.bass_utils import run_bass_kernel_spmd

B, H, L, D = 2, 16, 2048, 128
NCORES = 8
HPC = H // NCORES  # heads per core per batch
SLOTS = B * HPC  # bh slots per core
NEG = -1e9
INV_SQRT_D = 1.0 / math.sqrt(D)
F32 = mybir.dt.float32
F32R = mybir.dt.float32r
QT = L // 128  # 16 q tiles
QB = 4  # q blocks
QBW = L // QB  # 512 q per block
QTB = QT // QB  # 4 q tiles per block
EXPF = mybir.ActivationFunctionType.Exp

_cache: dict = {}


def _build(K0: int, K1: int):
    """Build+compile the per-core program for K0/K1 valid key tiles."""
    Ks = [K0, K0, K1, K1]
    KM = max(K0, K1)
    nc = bacc.Bacc("TRN2", target_bir_lowering=False, debug=False, num_devices=NCORES)
    q = nc.dram_tensor("q", [SLOTS, L, D], F32R, kind="ExternalInput")
    k = nc.dram_tensor("k", [SLOTS, KM * 128, D], F32R, kind="ExternalInput")
    v = nc.dram_tensor("v", [SLOTS, KM * 128, D], F32R, kind="ExternalInput")
    identr = nc.dram_tensor("identr", [128, 128], F32R, kind="ExternalInput")
    identf = nc.dram_tensor("identf", [128, 128], F32, kind="ExternalInput")
    onesr = nc.dram_tensor("onesr", [128, 1], F32R, kind="ExternalInput")
    onef = nc.dram_tensor("onef", [1, 1], F32, kind="ExternalInput")
    biases = nc.dram_tensor("biases", [128, SLOTS], F32, kind="ExternalInput")
    out = nc.dram_tensor("out", [SLOTS, L, D], F32, kind="ExternalOutput")

    # j pairs: all-but-last j grouped in twos, last j always alone (it takes
    # the mask bias)
    def jgroups(Kv):
        return [(j, 1) for j in range(Kv)]

    with tile.TileContext(nc) as tc:
        with (
            tc.tile_pool(name="const", bufs=1) as constp,
            tc.tile_pool(name="io", bufs=2) as iop,
            tc.tile_pool(name="work", bufs=3) as workp,
            tc.tile_pool(name="psst", bufs=4, space="PSUM") as psst,
            tc.tile_pool(name="pstr", bufs=2, space="PSUM") as pstr,
            tc.tile_pool(name="psac", bufs=1, space="PSUM") as psac,
            tc.tile_pool(name="dram", bufs=2, space="DRAM") as dramp,
        ):
            ident_r = constp.tile([128, 128], F32R)
            nc.sync.dma_start(out=ident_r, in_=identr[:, :])

            def emit_kv_loads(s):
                Kv = Ks[s]
                kn = iop.tile([128, KM, 128], F32R, tag="kn")
                nc.sync.dma_start(
                    out=kn[:, :Kv, :],
                    in_=k[s, : Kv * 128, :].rearrange("(t p) d -> p t d", p=128),
                )
                vn = iop.tile([128, KM, 128], F32R, tag="vn")
                nc.sync.dma_start(
                    out=vn[:, :Kv, :],
                    in_=v[s, : Kv * 128, :].rearrange("(t p) d -> p t d", p=128),
                )
                return kn, vn

            order0 = sorted(range(SLOTS), key=lambda x: -Ks[x])
            preload = {order0[0]: emit_kv_loads(order0[0])}
            qn0 = workp.tile([128, QTB, 128], F32R, tag="qn", bufs=5)
            nc.sync.dma_start(
                out=qn0,
                in_=q[order0[0], :QBW, :].rearrange("(p t) d -> p t d", p=128),
            )
            qn_preload = {(order0[0], 0): qn0}

            ident_f = constp.tile([128, 128], F32)
            nc.sync.dma_start(out=ident_f, in_=identf[:, :])
            ones_r = constp.tile([128, 1], F32R)
            nc.sync.dma_start(out=ones_r, in_=onesr[:, :])
            one_f = constp.tile([1, 1], F32)
            nc.sync.dma_start(out=one_f, in_=onef[:, :])
            bias_sb = constp.tile([128, SLOTS], F32)
            nc.sync.dma_start(out=bias_sb, in_=biases[:, :])

            def emit_finish(s, oT_slot, lrec):
                o_sb = workp.tile([128, QT, 128], F32, tag="o_sb", bufs=2)
                for g in range(QT // 4):
                    otr = pstr.tile([128, 4, 128], F32, tag="tr")
                    for ii in range(4):
                        nc.tensor.transpose(
                            otr[:, ii, :], oT_slot[:, g * 4 + ii, :], ident_f
                        )
                    for ii in range(4):
                        t = g * 4 + ii
                        nc.vector.tensor_scalar_mul(
                            o_sb[:, t, :], otr[:, ii, :], lrec[:, t : t + 1]
                        )
                nc.sync.dma_start(
                    out=out[s].rearrange("(b p t) d -> p b t d", p=128, t=QTB),
                    in_=o_sb.rearrange("p (b t) d -> p b t d", t=QTB),
                )

            def emit_qb_finish(s, qb, oT_slot, l_sbq):
                # tiny K=1 matmuls transpose l into per-partition layout
                ltq = pstr.tile([128, QTB], F32, tag="tr")
                for t in range(QTB):
                    nc.tensor.matmul(
                        ltq[:, t : t + 1],
                        l_sbq[:, t * 128 : (t + 1) * 128],
                        one_f[:, :],
                        start=(t == 0),
                        stop=(t == QTB - 1),
                        skip_group_check=True,
                    )
                lrecq = workp.tile([128, QTB], F32, tag="lrecq")
                nc.vector.reciprocal(lrecq, ltq)
                o_sbq = workp.tile([128, QTB, 128], F32, tag="o_sbq")
                otr = pstr.tile([128, 4, 128], F32, tag="tr")
                for ii in range(QTB):
                    nc.tensor.transpose(
                        otr[:, ii, :], oT_slot[:, qb * QTB + ii, :], ident_f
                    )
                for ii in range(QTB):
                    nc.vector.tensor_scalar_mul(
                        o_sbq[:, ii, :], otr[:, ii, :], lrecq[:, ii : ii + 1]
                    )
                nc.sync.dma_start(
                    out=out[s].rearrange("(b p t) d -> p b t d", p=128, t=QTB)[
                        :, qb, :, :
                    ],
                    in_=o_sbq,
                )

            pending = None
            pending_qb = None
            order = sorted(range(SLOTS), key=lambda x: -Ks[x])
            for idx, s in enumerate(order):
                Kv = Ks[s]
                is_last = idx == SLOTS - 1
                if s in preload:
                    kn, vn = preload.pop(s)
                else:
                    kn, vn = emit_kv_loads(s)
                # k -> kT [D, k]
                kTt = iop.tile([128, KM, 128], F32R, tag="kT")
                for g in range((Kv + 3) // 4):
                    n = min(4, Kv - g * 4)
                    trp = pstr.tile([128, 4, 128], F32R, tag="tr")
                    for ii in range(n):
                        nc.tensor.transpose(trp[:, ii, :], kn[:, g * 4 + ii, :], ident_r)
                    nc.scalar.copy(kTt[:, g * 4 : g * 4 + n, :], trp[:, :n, :])

                oT_slot = workp.tile([128, QT, 128], F32, tag="oT_slot", bufs=2)
                l_slot = workp.tile([1, L], F32, tag="l_slot")

                # issue all q-block loads for this slot upfront so the PE
                # never waits on DMA-issue jitter mid-slot
                qns = []
                for qb in range(QB):
                    if (s, qb) in qn_preload:
                        qns.append(qn_preload.pop((s, qb)))
                    else:
                        qn = workp.tile([128, QTB, 128], F32R, tag="qn", bufs=5)
                        nc.sync.dma_start(
                            out=qn,
                            in_=q[s, qb * QBW : (qb + 1) * QBW, :].rearrange(
                                "(p t) d -> p t d", p=128
                            ),
                        )
                        qns.append(qn)

                for qb in range(QB):
                    qn = qns[qb]
                    qTt = workp.tile([128, QTB, 128], F32R, tag="qT")
                    trp = pstr.tile([128, 4, 128], F32R, tag="tr")
                    for ii in range(QTB):
                        nc.tensor.transpose(trp[:, ii, :], qn[:, ii, :], ident_r)
                    nc.vector.tensor_copy(qTt, trp)

                    oT_ps = psac.tile([128, QBW], F32, tag="oT")
                    l_ps = psac.tile([1, QBW], F32, tag="l")
                    for (j0, npair) in jgroups(Kv):
                        st = psst.tile([128, 1, QBW], F32, tag="st")
                        for jj in range(npair):
                            nc.tensor.matmul(
                                st[:, jj, :],
                                kTt[:, j0 + jj, :],
                                qTt,
                                start=True,
                                stop=True,
                            )
                        pT = workp.tile([128, 1, QBW], F32R, tag="pT", bufs=6)
                        last = j0 + npair == Kv
                        nc.scalar.activation(
                            pT[:, :npair, :],
                            st[:, :npair, :],
                            EXPF,
                            bias=(bias_sb[:, s : s + 1] if last else 0.0),
                            scale=INV_SQRT_D,
                        )
                        for jj in range(npair):
                            j = j0 + jj
                            nc.tensor.matmul(
                                oT_ps,
                                vn[:, j, :],
                                pT[:, jj, :],
                                start=(j == 0),
                                stop=(j == Kv - 1),
                            )
                            nc.tensor.matmul(
                                l_ps,
                                ones_r,
                                pT[:, jj, :],
                                start=(j == 0),
                                stop=(j == Kv - 1),
                            )
                    nc.vector.tensor_copy(oT_slot[:, qb * QTB : (qb + 1) * QTB, :], oT_ps)
                    if not is_last:
                        nc.scalar.copy(l_slot[:, qb * QBW : (qb + 1) * QBW], l_ps)
                    else:
                        l_sbq = workp.tile([1, QBW], F32, tag="l_sbq")
                        nc.scalar.copy(l_sbq, l_ps)
                        if pending_qb is not None:
                            emit_qb_finish(*pending_qb)
                        pending_qb = (s, qb, oT_slot, l_sbq)
                        if qb == 0 and pending is not None:
                            emit_finish(*pending)
                            pending = None

                if not is_last:
                    lrec = workp.tile([128, QT], F32, tag="lrec")
                    # l: [1, 2048] -> DRAM bounce -> [q%128, q//128]
                    lrows = dramp.tile([1, L], F32, tag="lrows")
                    nc.sync.dma_start(out=lrows, in_=l_slot)
                    lcol = workp.tile([128, QT], F32, tag="lcol")
                    nc.sync.dma_start(
                        out=lcol,
                        in_=lrows[0, :].rearrange("(t p) -> p t", p=128),
                    )
                    nc.vector.reciprocal(lrec, lcol)
                    # defer the O^T -> O finish by one slot so its l-latency
                    # hides under the next slot's compute
                    if pending is not None:
                        emit_finish(*pending)
                    pending = (s, oT_slot, lrec)
            if pending is not None:
                emit_finish(*pending)
            if pending_qb is not None:
                emit_qb_finish(*pending_qb)
    nc.compile()
    return nc


def _get_program(K0: int, K1: int):
    key = (K0, K1)
    if key not in _cache:
        _cache[key] = _build(K0, K1)
    return _cache[key]


def _run(q, k, v, valid_lens, trace=False):
    q = np.ascontiguousarray(np.asarray(q, dtype=np.float32))
    k = np.ascontiguousarray(np.asarray(k, dtype=np.float32))
    v = np.ascontiguousarray(np.asarray(v, dtype=np.float32))
    vl = np.asarray(valid_lens).astype(np.int64)
    K0 = int(max(1, -(-vl[0] // 128)))
    K1 = int(max(1, -(-vl[1] // 128)))
    KM = max(K0, K1)
    nc = _get_program(K0, K1)

    # per-slot mask bias column: 0 for valid positions in the last key tile,
    # -1e9 beyond valid_len
    biases = np.zeros((128, SLOTS), dtype=np.float32)
    Ks = [K0, K0, K1, K1]
    bs = [0, 0, 1, 1]
    pos = np.arange(128)
    for s in range(SLOTS):
        rem = int(vl[bs[s]]) - (Ks[s] - 1) * 128
        biases[:, s] = np.where(pos < rem, 0.0, np.float32(NEG))

    identf = np.eye(128, dtype=np.float32)
    onesr = np.ones((128, 1), dtype=np.float32)

    in_maps = []
    for c in range(NCORES):
        h0, h1 = 2 * c, 2 * c + 1
        qs = np.ascontiguousarray(
            np.stack([q[0, h0], q[0, h1], q[1, h0], q[1, h1]])
        )
        ks = np.ascontiguousarray(
            np.stack(
                [
                    k[0, h0, : KM * 128],
                    k[0, h1, : KM * 128],
                    k[1, h0, : KM * 128],
                    k[1, h1, : KM * 128],
                ]
            )
        )
        vs = np.ascontiguousarray(
            np.stack(
                [
                    v[0, h0, : KM * 128],
                    v[0, h1, : KM * 128],
                    v[1, h0, : KM * 128],
                    v[1, h1, : KM * 128],
                ]
            )
        )
        in_maps.append(
            {
                "q": qs,
                "k": ks,
                "v": vs,
                "identr": identf,
                "identf": identf,
                "onesr": onesr,
                "onef": onesr[:1, :1],
                "biases": biases,
            }
        )

    try:
        res = run_bass_kernel_spmd(
            nc, in_maps, core_ids=list(range(NCORES)), trace=trace
        )
    except Exception:
        # transient device wedges (NRT_EXEC_UNIT_UNRECOVERABLE) have been
        # observed to clear on retry
        res = run_bass_kernel_spmd(
            nc, in_maps, core_ids=list(range(NCORES)), trace=trace
        )

    outp = np.empty((B, H, L, D), dtype=np.float32)
    for c in range(NCORES):
        o = res.results[c]["out"]
        h0, h1 = 2 * c, 2 * c + 1
        outp[0, h0] = o[0]
        outp[0, h1] = o[1]
        outp[1, h0] = o[2]
        outp[1, h1] = o[3]
    return outp, res


def kernel(q, k, v, valid_lens):
    outp, _ = _run(q, k, v, valid_lens, trace=False)
    return outp



# revision 6
# speedup vs baseline: 1.4016x; 1.4016x over previous
"""Masked dot-product attention (B=2,H=16,L=2048,D=128) on 8 trn2 NeuronCores.

Strategy (v2):
  - Shard batch*heads: core c handles (b=0,h=2c),(0,2c+1),(1,2c),(1,2c+1)
    -> 4 slots, so every core carries one K0-slot pair and one K1-slot pair
    (balanced work).
  - Host pre-transposes q and k per slot into [D, L] / [D, Kv*128] bf16 and
    pre-permutes v into its SBUF image [128, Kv, 130] bf16 with a ones column
    at d=128 -> zero on-device transposes, fully contiguous DMAs.
  - Scores: S^T[k, q] = matmul(lhsT=kT_j, rhs=qT-block) in bf16
    (1 cycle/row).  Masking is a per-partition bias on the exp of the last
    key tile only.
  - exp fused into PSUM->SBUF eviction on the Act engine with
    scale=1/sqrt(D), j-pairs share one instruction; pT evicted as bf16.
  - PV: O[q, d] computed in natural layout via
    matmul(out[128q, 129], lhsT=pT[:, jj, qtile], rhs=[V_j | ones]);
    the appended ones column accumulates the softmax denominator l per
    q-partition for free (no [1,N] l-matmul, no transposes of O or l).
  - Finish per q-block: DVE reciprocal of the l column + 4 per-partition
    scalar muls, then one contiguous 256KB DMA of the fp32 output.
  - Software pipelining: scores for group g+1 are emitted before PV of
    group g so the in-order PE queue never head-of-line blocks on the Act
    engine; st/pT/o_ps pools are double-buffered (8 PSUM banks exactly).
"""

import math

import numpy as np

try:
    import concourse.bass as bass
except ImportError:  # pragma: no cover
    import sys

    sys.path.append("/opt/trn_rl_repo")
    import concourse.bass as bass

import ml_dtypes
import concourse.mybir as mybir
import concourse.tile as tile
from concourse import bacc
from concourse.bass_utils import run_bass_kernel_spmd

B, H, L, D = 2, 16, 2048, 128
NCORES = 8
HPC = H // NCORES  # heads per core per batch
SLOTS = B * HPC  # bh slots per core
NEG = -1e9
INV_SQRT_D = 1.0 / math.sqrt(D)
F32 = mybir.dt.float32
BF16 = mybir.dt.bfloat16
QB = 4  # q blocks per slot
QBW = L // QB  # 512 q per block
QTB = QBW // 128  # 4 q tiles per block
VW = 130  # v tile width: 128 d + ones col + pad
EXPF = mybir.ActivationFunctionType.Exp
NPBF16 = np.dtype(ml_dtypes.bfloat16)

_cache: dict = {}


def _jgroups(Kv):
    """j in pairs, the last j always alone (it takes the mask bias)."""
    out = []
    j = 0
    while j < Kv - 1:
        n = 2 if j + 2 <= Kv - 1 else 1
        out.append((j, n))
        j += n
    out.append((Kv - 1, 1))
    return out


def _build(K0: int, K1: int):
    """Build+compile the per-core program for K0/K1 valid key tiles."""
    Ks = [K0, K0, K1, K1]
    KM = max(K0, K1)
    nc = bacc.Bacc("TRN2", target_bir_lowering=False, debug=False, num_devices=NCORES)
    qT = nc.dram_tensor("qT", [SLOTS, 128, L], BF16, kind="ExternalInput")
    kT = nc.dram_tensor("kT", [SLOTS, 128, KM * 128], BF16, kind="ExternalInput")
    vp = nc.dram_tensor("vp", [SLOTS, 128, KM * VW], BF16, kind="ExternalInput")
    biases = nc.dram_tensor("biases", [128, SLOTS], F32, kind="ExternalInput")
    out = nc.dram_tensor("out", [SLOTS, L, D], F32, kind="ExternalOutput")

    order = sorted(range(SLOTS), key=lambda x: -Ks[x])

    with tile.TileContext(nc) as tc:
        with (
            tc.tile_pool(name="const", bufs=1) as constp,
            tc.tile_pool(name="io", bufs=1) as iop,
            tc.tile_pool(name="pt", bufs=3) as ptp,
            tc.tile_pool(name="fin", bufs=3) as finp,
            tc.tile_pool(name="psst", bufs=2, space="PSUM") as psst,
            tc.tile_pool(name="psoa", bufs=1, space="PSUM") as psoa,
        ):
            bias_sb = constp.tile([128, SLOTS], F32)
            nc.sync.dma_start(out=bias_sb, in_=biases[:, :])

            # preload all slot inputs upfront (SBUF easily fits them);
            # k/v first for the first slot so compute can start early,
            # q in per-block chunks so the first matmul doesn't wait on
            # the whole 512KB load
            kts, vps, qts = {}, {}, {}
            s0 = order[0]
            kts[s0] = constp.tile([128, KM * 128], BF16, tag=f"kt{s0}", name=f"kt{s0}")
            nc.sync.dma_start(out=kts[s0][:, : Ks[s0] * 128], in_=kT[s0, :, : Ks[s0] * 128])
            vps[s0] = constp.tile([128, KM, VW], BF16, tag=f"vp{s0}", name=f"vp{s0}")
            nc.sync.dma_start(
                out=vps[s0][:, : Ks[s0], :],
                in_=vp[s0, :, : Ks[s0] * VW].rearrange("p (t w) -> p t w", w=VW),
            )
            qts[s0] = constp.tile([128, L], BF16, tag=f"qt{s0}", name=f"qt{s0}")
            for qb in range(QB):
                nc.sync.dma_start(
                    out=qts[s0][:, qb * QBW : (qb + 1) * QBW],
                    in_=qT[s0, :, qb * QBW : (qb + 1) * QBW],
                )
            for s in order[1:]:
                Kv = Ks[s]
                kts[s] = constp.tile([128, KM * 128], BF16, tag=f"kt{s}", name=f"kt{s}")
                nc.sync.dma_start(out=kts[s][:, : Kv * 128], in_=kT[s, :, : Kv * 128])
                vps[s] = constp.tile([128, KM, VW], BF16, tag=f"vp{s}", name=f"vp{s}")
                nc.sync.dma_start(
                    out=vps[s][:, :Kv, :],
                    in_=vp[s, :, : Kv * VW].rearrange("p (t w) -> p t w", w=VW),
                )
                qts[s] = constp.tile([128, L], BF16, tag=f"qt{s}", name=f"qt{s}")
                nc.sync.dma_start(out=qts[s], in_=qT[s])

            for s in order:
                Kv = Ks[s]
                groups = _jgroups(Kv)
                kt_sb, vp_sb, qt_sb = kts[s], vps[s], qts[s]
                for qb in range(QB):
                    qs = qt_sb[:, qb * QBW : (qb + 1) * QBW]
                    # one PSUM bank per q-tile accumulator (the PE cannot
                    # interleave two accumulation regions within one bank);
                    # col 128 accumulates l via the ones column of vp
                    o_ps = psoa.tile([128, QTB, 512], F32, tag="o_ps")

                    # emit score matmuls one group ahead of PV so the
                    # in-order PE queue never waits on the Act engine
                    sts = [None] * len(groups)

                    def emit_scores(g):
                        j0, npair = groups[g]
                        st = psst.tile([128, npair, QBW], F32, tag="st", name="st")
                        for jj in range(npair):
                            nc.tensor.matmul(
                                st[:, jj, :],
                                kt_sb[:, (j0 + jj) * 128 : (j0 + jj + 1) * 128],
                                qs,
                                start=True,
                                stop=True,
                            )
                        sts[g] = st

                    emit_scores(0)
                    for g, (j0, npair) in enumerate(groups):
                        st = sts[g]
                        last = j0 + npair == Kv
                        pT = ptp.tile([128, npair, QBW], BF16, tag="pT")
                        nc.scalar.activation(
                            pT,
                            st[:, :npair, :],
                            EXPF,
                            bias=(bias_sb[:, s : s + 1] if last else 0.0),
                            scale=INV_SQRT_D,
                        )
                        if g + 1 < len(groups):
                            emit_scores(g + 1)
                        for jj in range(npair):
                            j = j0 + jj
                            for qt in range(QTB):
                                nc.tensor.matmul(
                                    o_ps[:, qt, : D + 1],
                                    pT[:, jj, qt * 128 : (qt + 1) * 128],
                                    vp_sb[:, j, : D + 1],
                                    start=(j == 0),
                                    stop=(j == Kv - 1),
                                    skip_group_check=True,
                                )

                    # fine-grained finish: free each o_ps bank as soon as
                    # its q-tile's accumulation stops, so the next q-block's
                    # PV matmuls overlap with this finish
                    lrec = finp.tile([128, QTB], F32, tag="lrec")
                    o_sb = finp.tile([128, QTB, 128], F32, tag="o_sb")
                    for qt in range(QTB):
                        nc.vector.reciprocal(
                            lrec[:, qt : qt + 1], o_ps[:, qt, D : D + 1]
                        )
                        nc.vector.tensor_scalar_mul(
                            o_sb[:, qt, :],
                            o_ps[:, qt, :D],
                            lrec[:, qt : qt + 1],
                        )
                    nc.sync.dma_start(
                        out=out[s].rearrange("(b t p) d -> p b t d", p=128, t=QTB)[
                            :, qb
                        ],
                        in_=o_sb,
                    )
    nc.compile()
    return nc


def _get_program(K0: int, K1: int):
    key = (K0, K1)
    if key not in _cache:
        _cache[key] = _build(K0, K1)
    return _cache[key]


def _run(q, k, v, valid_lens, trace=False):
    q = np.asarray(q, dtype=np.float32)
    k = np.asarray(k, dtype=np.float32)
    v = np.asarray(v, dtype=np.float32)
    vl = np.asarray(valid_lens).astype(np.int64)
    K0 = int(max(1, -(-vl[0] // 128)))
    K1 = int(max(1, -(-vl[1] // 128)))
    KM = max(K0, K1)
    nc = _get_program(K0, K1)

    # per-slot mask bias column: 0 for valid positions in the last key tile,
    # -1e9 beyond valid_len
    biases = np.zeros((128, SLOTS), dtype=np.float32)
    Ks = [K0, K0, K1, K1]
    bs = [0, 0, 1, 1]
    pos = np.arange(128)
    for s in range(SLOTS):
        rem = int(vl[bs[s]]) - (Ks[s] - 1) * 128
        biases[:, s] = np.where(pos < rem, 0.0, np.float32(NEG))

    # host-side prep: [B,H,L,D] fp32 -> per-slot transposed bf16 images
    qb16 = q.astype(NPBF16)  # [B,H,L,D]
    kb16 = k.astype(NPBF16)
    vb16 = v.astype(NPBF16)

    in_maps = []
    for c in range(NCORES):
        h0, h1 = 2 * c, 2 * c + 1
        bh = [(0, h0), (0, h1), (1, h0), (1, h1)]
        qTs = np.empty((SLOTS, 128, L), dtype=NPBF16)
        kTs = np.zeros((SLOTS, 128, KM * 128), dtype=NPBF16)
        vps = np.zeros((SLOTS, 128, KM * VW), dtype=NPBF16)
        for s, (b, h) in enumerate(bh):
            qTs[s] = qb16[b, h].T
            Kv = Ks[s]
            kTs[s, :, : Kv * 128] = kb16[b, h, : Kv * 128].T
            # v SBUF image: [p, t, w]: w<128 -> v[t*128+p, w]; w==128 -> 1
            vt = np.zeros((128, Kv, VW), dtype=NPBF16)
            vt[:, :, :128] = vb16[b, h, : Kv * 128].reshape(Kv, 128, 128).transpose(
                1, 0, 2
            )
            vt[:, :, 128] = NPBF16.type(1.0)
            vps[s, :, : Kv * VW] = vt.reshape(128, Kv * VW)
        in_maps.append(
            {
                "qT": np.ascontiguousarray(qTs),
                "kT": np.ascontiguousarray(kTs),
                "vp": np.ascontiguousarray(vps),
                "biases": biases,
            }
        )

    try:
        res = run_bass_kernel_spmd(
            nc, in_maps, core_ids=list(range(NCORES)), trace=trace
        )
    except Exception:
        # transient device wedges (NRT_EXEC_UNIT_UNRECOVERABLE) have been
        # observed to clear on retry
        res = run_bass_kernel_spmd(
            nc, in_maps, core_ids=list(range(NCORES)), trace=trace
        )

    outp = np.empty((B, H, L, D), dtype=np.float32)
    for c in range(NCORES):
        o = res.results[c]["out"]
        h0, h1 = 2 * c, 2 * c + 1
        outp[0, h0] = o[0]
        outp[0, h1] = o[1]
        outp[1, h0] = o[2]
        outp[1, h1] = o[3]
    return outp, res


def kernel(q, k, v, valid_lens):
    outp, _ = _run(q, k, v, valid_lens, trace=False)
    return outp
